# revision 35
# baseline (speedup 1.0000x reference)
"""Distributed Trainium2 Bass kernel for nn_AdjGNN (GNN message passing).

Strategy (8 NeuronCores, SPMD):
  - Shard nodes (and their incident edges via dst ownership) evenly: core c
    owns nodes [2500c, 2500c+2500).
  - Per GCN layer: each core computes h = z @ (trans_W @ emb_W_next) for its
    nodes (weights folded host-side), casts to bf16 and AllGathers h across
    the 8 cores into a per-core DRAM replica (halo exchange; src indices are
    uniform-random so the halo is the full node set).  The x-residual
    transform (x' = z @ trans_W) is deferred into the AllGather window so the
    collective starts as early as possible and the PE keeps busy during it.
  - segment_sum(h[src], dst): per-edge rows are fetched with dma_gather
    (one 512B descriptor per edge, four SWDGE queues) into SBUF laid out so
    block k holds the k-th edge of 256 consecutive dsts; the 16:1 in-degree
    reduction is 16 PSUM-accumulating identity matmuls on the TensorEngine
    (bf16 in, fp32 accumulate).  Chunks are fetched as 2 (first chunks: 4)
    sub-gathers so the PE can start reducing before the whole chunk lands.
  - Degree norm + ReLU on ACT; weighted residual fused into one DVE op;
    transform matmuls run in bf16 from PE-transposed z.
  - Readout uses linearity: mean_nodes(z @ trans_W) = (P @ z) @ trans_W, so
    the last layer only accumulates (P @ z)^T per chunk ([256, 100]); the
    final transform runs once on the AllReduced [256, 100] result, then the
    small MLP head (bf16) on every core.

The graph structure (src/dst/graph_id) is known when kernel() is called, so
all index tables (gather indices, inverse degrees, readout matrix) and the
layer weight folds are precomputed host-side.
"""

import ml_dtypes
import numpy as np

import concourse.bacc as bacc
import concourse.mybir as mybir
import concourse.tile as tile
from concourse.bass_utils import run_bass_kernel_spmd

FP = mybir.dt.float32
BF = mybir.dt.bfloat16
F8 = mybir.dt.float8e4
I16 = mybir.dt.int16
AF = mybir.ActivationFunctionType
ALU = mybir.AluOpType

N, E, G = 20000, 320000, 100
IN, H, L = 74, 256, 5
M1, M2 = 1024, 512
NEG = 0.01
C = 8
NLOC = N // C              # 2500 nodes per core
NT = (NLOC + 127) // 128   # 20 node tiles per core
NPAD = NT * 128            # 2560
CH = NT // 2               # chunks of 256 dsts
NFULL = NPAD * C           # 20480

_CACHE = {}
L4_TRICK = True   # readout via (P z) @ W after AllReduce
AG_PIECES = 1     # split each AllGather into row-pieces (1 = single Shared AG)
PIECE = NPAD // AG_PIECES
AG_TRIG_CH = 8    # chunk before whose gathers the first piece is triggered
FEAT_F8 = True    # h features in fp8-e4m3 (halves gather + AllGather bytes)
HT = F8 if FEAT_F8 else BF


def _host_prep(src, dst, gid):
    """Index-only host preprocessing -> per-core gather tables etc."""
    src = np.asarray(src).astype(np.int64)
    dst = np.asarray(dst).astype(np.int64)
    gid = np.asarray(gid).astype(np.int64)

    order = np.argsort(dst, kind="stable")
    src_s, dst_s = src[order], dst[order]
    counts = np.bincount(dst_s, minlength=N)
    starts = np.zeros(N + 1, np.int64)
    np.cumsum(counts, out=starts[1:])
    K = int(counts.max())

    # h_full layout: [piece, rank, PIECE rows] so each AllGather piece is a
    # contiguous block.  Pad gather slots point at row 0 (results for those
    # dsts are zeroed by scl=0, garbage is fine).
    loc = src_s % NLOC
    grow_src = (loc // PIECE) * (C * PIECE) + (src_s // NLOC) * PIECE + (loc % PIECE)

    idx_tables = []
    for c in range(C):
        flat = np.zeros(CH * K * 256, np.int64)
        base = c * NLOC
        for l in range(NLOC):
            n = base + l
            s0, d = starts[n], counts[n]
            ch, r = l // 256, l % 256
            pos0 = ch * K * 256 + r
            flat[pos0 : pos0 + 256 * d : 256] = grow_src[s0 : s0 + d]
        w = flat.reshape(-1, 16).T.astype(np.int16)   # [16, CH*K*16]
        idx_tables.append(np.ascontiguousarray(np.tile(w, (8, 1))))

    inv = np.zeros((C, 128, NT), np.float64)
    for c in range(C):
        cnt = counts[c * NLOC : (c + 1) * NLOC].astype(np.float64)
        iv = np.zeros(NPAD, np.float64)
        iv[:NLOC] = 1.0 / np.maximum(cnt, 1.0)
        inv[c] = iv.reshape(NT, 128).T

    gcnt = np.bincount(gid, minlength=G).astype(np.float64)
    gcnt = np.maximum(gcnt, 1.0)
    P = np.zeros((C, 128, NT, G), np.float32)
    for c in range(C):
        for l in range(NLOC):
            n = c * NLOC + l
            P[c, l % 128, l // 128, gid[n]] = 1.0 / gcnt[gid[n]]

    return idx_tables, inv, P, K, counts


def _build(K, uniform_deg, w0_sign, w1_vals):
    """Build the Bacc graph. w0_sign[i]: +1 fold w0 into relu scale, else 0.
    w1_vals: flattened sum_w baked as immediates."""
    nc = bacc.Bacc("TRN2", debug=False, num_devices=C, num_swdge_queues=4)

    nh_p = nc.declare_dram_parameter("node_hT", [IN, NPAD], BF, isOutput=False)
    idx_p = nc.declare_dram_parameter("gidx", [128, CH * K * 16], I16, isOutput=False)
    scl_p = nc.declare_dram_parameter("scl", [128, L * NT], FP, isOutput=False)
    P_p = nc.declare_dram_parameter("Pmat", [128, NT, G], BF, isOutput=False)
    id_p = nc.declare_dram_parameter("ident", [128, 128], FP, isOutput=False)
    nemb_p = nc.declare_dram_parameter("nembW", [IN, H], BF, isOutput=False)
    w0_p = nc.declare_dram_parameter("W0", [IN, H], BF, isOutput=False)
    wc_p = nc.declare_dram_parameter("wcW", [L - 1, 128, 2, H], BF, isOutput=False)
    trn_p = nc.declare_dram_parameter("trnW", [L, 128, 2, H], FP, isOutput=False)
    w1_p = nc.declare_dram_parameter("mlpW1", [128, 2, M1], FP, isOutput=False)
    w2_p = nc.declare_dram_parameter("mlpW2", [128, 8, M2], FP, isOutput=False)
    w3_p = nc.declare_dram_parameter("mlpW3", [128, 4, 1], FP, isOutput=False)
    out_p = nc.declare_dram_parameter("out", [1, G], FP, isOutput=True)

    with tile.TileContext(nc) as tc:
        with (
            tc.tile_pool(name="dram", bufs=1, space="DRAM") as dram,
            tc.tile_pool(name="sb", bufs=1) as sb,
            tc.tile_pool(name="sb2", bufs=2) as sb2,
            tc.tile_pool(name="ps", bufs=2, space="PSUM") as ps,
        ):
            ag_in = [dram.tile([NPAD, H], HT, name=f"ag_in{i}") for i in range(L)]
            h_space = {"addr_space": "Shared"} if AG_PIECES == 1 else {}
            h_full = [
                dram.tile([NFULL, H], HT, name=f"h_full{i}", **h_space)
                for i in range(L)
            ]
            if L4_TRICK:
                ar_in = dram.tile([128, 2 * G], FP)
                ar_out = dram.tile([128, 2 * G], FP, addr_space="Shared")
            else:
                ar_in = dram.tile([G, H], FP)
                ar_out = dram.tile([G, H], FP, addr_space="Shared")

            ident = sb.tile([128, 128], FP)
            identb = sb.tile([128, 128], BF)
            identh = sb.tile([128, 128], HT)
            idxs = sb.tile([128, CH * K * 16], I16)
            scl = sb.tile([128, L * NT], FP)
            nembb = sb.tile([IN, H], BF)
            w0b = sb.tile([IN, H], BF)
            tw = [sb.tile([128, 2, H], FP, name=f"tw{i}") for i in range(L)]
            wc = [sb.tile([128, 2, H], BF, name=f"wc{i}") for i in range(L - 1)]
            nhT = sb.tile([IN, NT, 128], BF)
            x_a = sb.tile([128, NT, H], FP)
            x_b = sb.tile([128, NT, H], FP)
            zTall = sb.tile([128, CH, 2, H], BF)
            w1s = sb.tile([128, 2, M1], FP)
            w2s = sb.tile([128, 8, M2], FP)
            w3s = sb.tile([128, 4, 1], FP)
            Psb = sb.tile([128, NT, G], BF)
            if not L4_TRICK:
                Psf = sb.tile([128, NT, G], FP)
                gsum_s = sb.tile([G, H], FP)
                gmean = sb.tile([G, H], FP)
            gzs = sb.tile([128, 2, G], FP)
            arT = sb.tile([128, 2, G], FP)
            gT = sb.tile([128, 2, G], FP)
            h1T = sb.tile([128, 8, G], FP)
            h2T = sb.tile([128, 4, G], FP)
            outs = sb.tile([1, G], FP)

            nc.sync.dma_start(ident[:, :], id_p[:, :])
            nc.scalar.activation(identb[:, :], ident[:, :], AF.Copy)
            nc.scalar.activation(identh[:, :], ident[:, :], AF.Copy)
            nc.sync.dma_start(idxs[:, :], idx_p[:, :])
            nc.sync.dma_start(scl[:, :], scl_p[:, :])
            nc.sync.dma_start(w3s[:, :, :], w3_p[:, :, :])
            # bf16 weights arrive host-precast; plain HWDGE loads
            nc.sync.dma_start(
                nhT[:, :, :].rearrange("p t f -> p (t f)"), nh_p[:, :]
            )
            nc.sync.dma_start(w0b[:, :], w0_p[:, :])
            nc.sync.dma_start(nembb[:, :], nemb_p[:, :])
            nc.sync.dma_start(Psb[:, :, :], P_p[:, :, :])
            for i in range(L):
                nc.sync.dma_start(tw[i][:, :, :], trn_p[i, :, :, :])
            for i in range(L - 1):
                nc.sync.dma_start(wc[i][:, :, :], wc_p[i, :, :, :])
            nc.sync.dma_start(w1s[:, :, :], w1_p[:, :, :])
            nc.sync.dma_start(w2s[:, :, :], w2_p[:, :, :])

            # pad rows [NLOC, NPAD) of each ag_in are shipped by the AllGather
            # but never written by the layer body — zero them once
            zb = sb.tile([128, H], HT)
            nc.vector.memset(zb[:, :], 0.0)
            for i in range(L):
                nc.sync.dma_start(ag_in[i][NLOC:NPAD, :], zb[0 : NPAD - NLOC, :])

            # ---- front: h0 = node_h @ (node_emb_W @ emb_W[0]) per tile
            # (node_h arrives host-pretransposed as [IN, NPAD])
            for t in range(NT):
                rows = min(128, NLOC - t * 128)
                h0 = ps.tile([128, H], FP, tag="mm" if t % 2 else "msg")
                nc.tensor.matmul(h0[:, :], nhT[:, t, :], w0b[:, :], start=True, stop=True)
                hs = sb2.tile([128, H], HT, tag="hst")
                nc.scalar.activation(hs[:, :], h0[:, :], AF.Copy)
                nc.sync.dma_start(ag_in[0][t * 128 : t * 128 + rows, :], hs[0:rows, :])
                if (t + 1) % (PIECE // 128) == 0:
                    p = (t + 1) // (PIECE // 128) - 1
                    nc.gpsimd.collective_compute(
                        "AllGather", ALU.bypass, replica_groups=[list(range(C))],
                        ins=[ag_in[0][p * PIECE : (p + 1) * PIECE, :].opt()],
                        outs=[
                            h_full[0][p * PIECE * C : (p + 1) * PIECE * C, :].opt()
                        ],
                    )

            # x0 = node_h @ node_emb_W (overlaps the first AllGather)
            for t in range(NT):
                x0 = ps.tile([128, H], FP, tag="mm" if t % 2 else "msg")
                nc.tensor.matmul(x0[:, :], nhT[:, t, :], nembb[:, :], start=True, stop=True)
                nc.scalar.activation(x_a[:, t, :], x0[:, :], AF.Copy)

            x_cur, x_nxt = x_a, x_b
            gz = None
            gq = 0
            for i in range(L):
                last = i == L - 1
                # -- loop 1: everything that gates the next AllGather --------
                for ch in range(CH):
                    t0 = 2 * ch
                    if i < L - 1 and AG_PIECES == 2 and ch == AG_TRIG_CH:
                        nc.gpsimd.collective_compute(
                            "AllGather", ALU.bypass,
                            replica_groups=[list(range(C))],
                            ins=[ag_in[i + 1][0:PIECE, :].opt()],
                            outs=[h_full[i + 1][0 : PIECE * C, :].opt()],
                        )
                    # sub-split gathers: first chunks 4-way so the PE reduction
                    # starts early after the layer's h_full is ready
                    nsub = 4 if ch < 2 else 2
                    span = K // nsub
                    gbs = []
                    for sg in range(nsub):
                        if nsub == 2:
                            gsub = sb2.tile(
                                [128, span * 2 * H], HT, tag=f"gh{sg}", bufs=3
                            )
                        else:
                            gsub = sb2.tile(
                                [128, span * 2 * H], HT, tag=f"gs{sg}", bufs=1
                            )
                        nc.gpsimd.dma_gather(
                            gsub[:, 0 : span * 2 * H]
                            .rearrange("p (b e) -> p b e", e=H),
                            h_full[i][:, :],
                            idxs[:, (ch * K + sg * span) * 16 : (ch * K + (sg + 1) * span) * 16],
                            span * 256, span * 256, H,
                            single_packet=False,
                            queue_num=gq % 4,
                        )
                        gq += 1
                        gbs.append(gsub)
                    # PE reduces edge-blocks 0..NPE-1; DVE pairwise-sums the
                    # rest into a bf16 partial merged by one identity matmul
                    NPE = K // 2
                    msg = ps.tile([128, 2 * H], FP, tag="msg")
                    for k in range(NPE):
                        gsub = gbs[k // span]
                        kk = k % span
                        nc.tensor.matmul(
                            msg[:, :], identh[:, :],
                            gsub[:, 2 * kk * H : (2 * kk + 2) * H],
                            start=(k == 0), stop=False,
                        )

                    def blk(k):
                        g = gbs[k // span]
                        kk = k % span
                        return g[:, 2 * kk * H : (2 * kk + 2) * H]

                    pA = sb2.tile([128, 2 * H], BF, tag="pA")
                    pB = sb2.tile([128, 2 * H], BF, tag="pB")
                    nc.vector.tensor_add(pA[:, :], blk(NPE), blk(NPE + 1))
                    nc.vector.tensor_add(pB[:, :], blk(NPE + 2), blk(NPE + 3))
                    nc.vector.tensor_add(pA[:, :], pA[:, :], pB[:, :])
                    nc.vector.tensor_add(pB[:, :], blk(NPE + 4), blk(NPE + 5))
                    nc.vector.tensor_add(pA[:, :], pA[:, :], pB[:, :])
                    nc.vector.tensor_add(pB[:, :], blk(NPE + 6), blk(NPE + 7))
                    nc.vector.tensor_add(pA[:, :], pA[:, :], pB[:, :])
                    nc.tensor.matmul(
                        msg[:, :], identb[:, :], pA[:, :],
                        start=False, stop=True,
                    )
                    r = sb2.tile([128, 2, H], FP, tag="rt")
                    if uniform_deg:
                        nc.scalar.activation(
                            r[:, :, :], msg[:, :].rearrange("p (s e) -> p s e", e=H),
                            AF.Relu,
                            scale=scl[:, i * NT + t0 : i * NT + t0 + 1],
                        )
                    else:
                        for s in range(2):
                            nc.scalar.activation(
                                r[:, s, :], msg[:, s * H : (s + 1) * H],
                                AF.Relu,
                                scale=scl[:, i * NT + t0 + s : i * NT + t0 + s + 1],
                            )
                    if w0_sign[i] <= 0:
                        nc.vector.tensor_scalar_mul(r[:, :, :], r[:, :, :], w1_vals[2 * i])
                    # z = x*w1 + r fused on DVE, bf16 out for the transform path
                    z = sb2.tile([128, 2, H], BF, tag="zt")
                    nc.vector.scalar_tensor_tensor(
                        z[:, :, :], x_cur[:, t0 : t0 + 2, :], w1_vals[2 * i + 1],
                        r[:, :, :], ALU.mult, ALU.add,
                    )
                    if last and L4_TRICK:
                        # readout: accumulate (P @ z)^T = z^T P directly; the
                        # trans_W transform is applied after the AllReduce
                        if gz is None:
                            gz = [
                                ps.tile([128, G], FP, tag=f"gz{f}", bufs=1,
                                        name=f"gz{f}")
                                for f in range(2)
                            ]
                        for s in range(2):
                            for f in range(2):
                                nc.tensor.matmul(
                                    gz[f][:, :],
                                    z[:, s, f * 128 : (f + 1) * 128],
                                    Psb[:, t0 + s, :],
                                    start=(ch == 0 and s == 0),
                                    stop=(ch == CH - 1 and s == 1),
                                    skip_group_check=True,
                                )
                        continue
                    for s in range(2):
                        for f in range(2):
                            tpb = ps.tile([128, 128], BF, tag="tp")
                            nc.tensor.transpose(
                                tpb[:, :], z[:, s, f * 128 : (f + 1) * 128],
                                identb[:, :],
                            )
                            nc.vector.tensor_copy(
                                zTall[:, ch, f, s * 128 : (s + 1) * 128], tpb[:, :]
                            )
                    if not last:
                        hn = ps.tile([128, 2 * H], FP, tag="mm")
                        for s in range(2):
                            for f in range(2):
                                nc.tensor.matmul(
                                    hn[:, s * H : (s + 1) * H],
                                    zTall[:, ch, f, s * 128 : (s + 1) * 128],
                                    wc[i][:, f, :], start=(f == 0), stop=(f == 1),
                                )
                        hs = sb2.tile([128, 2 * H], HT, tag="hst")
                        nc.scalar.activation(hs[:, :], hn[:, :], AF.Copy)
                        for s in range(2):
                            t = t0 + s
                            rows = min(128, NLOC - t * 128)
                            if rows > 0:
                                nc.sync.dma_start(
                                    ag_in[i + 1][t * 128 : t * 128 + rows, :],
                                    hs[0:rows, s * H : (s + 1) * H],
                                )
                    else:
                        # L4_TRICK=False: baseline-style x transform + readout
                        xn = ps.tile([128, 2 * H], FP, tag="mm")
                        for s in range(2):
                            for f in range(2):
                                nc.tensor.matmul(
                                    xn[:, s * H : (s + 1) * H],
                                    zTall[:, ch, f, s * 128 : (s + 1) * 128],
                                    tw[i][:, f, :], start=(f == 0), stop=(f == 1),
                                )
                        nc.scalar.activation(x_nxt[:, t0 : t0 + 2, :], xn[:, :], AF.Copy)
                        if gz is None:
                            gz = ps.tile([G, H], FP, tag="gz", bufs=1)
                        for s in range(2):
                            nc.tensor.matmul(
                                gz[:, :], Psf[:, t0 + s, :], x_nxt[:, t0 + s, :],
                                start=(ch == 0 and s == 0),
                                stop=(ch == CH - 1 and s == 1),
                                skip_group_check=True,
                            )
                if last:
                    break
                p0 = PIECE * (AG_PIECES - 1)
                nc.gpsimd.collective_compute(
                    "AllGather", ALU.bypass, replica_groups=[list(range(C))],
                    ins=[ag_in[i + 1][p0:NPAD, :].opt()],
                    outs=[h_full[i + 1][p0 * C : NFULL, :].opt()],
                )
                # -- loop 2: x' = z @ trans_W, runs inside the AllGather window
                for ch in range(CH):
                    t0 = 2 * ch
                    zT32 = sb2.tile([128, 2, H], FP, tag="z32")
                    for f in range(2):
                        nc.scalar.activation(
                            zT32[:, f, :], zTall[:, ch, f, :], AF.Copy
                        )
                    xn = ps.tile([128, 2 * H], FP, tag="mm")
                    for s in range(2):
                        for f in range(2):
                            nc.tensor.matmul(
                                xn[:, s * H : (s + 1) * H],
                                zT32[:, f, s * 128 : (s + 1) * 128],
                                tw[i][:, f, :], start=(f == 0), stop=(f == 1),
                            )
                    nc.scalar.activation(x_nxt[:, t0 : t0 + 2, :], xn[:, :], AF.Copy)
                x_cur, x_nxt = x_nxt, x_cur

            # ---- readout tail
            if L4_TRICK:
                for f in range(2):
                    nc.scalar.activation(gzs[:, f, :], gz[f][:, :], AF.Copy)
                nc.sync.dma_start(ar_in[:, :], gzs[:, :, :])
                nc.gpsimd.collective_compute(
                    "AllReduce", ALU.add, replica_groups=[list(range(C))],
                    ins=[ar_in[:, :].opt()], outs=[ar_out[:, :].opt()],
                )
                nc.sync.dma_start(arT[:, :, :], ar_out[:, :])
                # gmean^T = trans_W[L-1]^T @ (P z)^T
                for m in range(2):
                    gm = ps.tile([128, G], FP, tag="mm")
                    for f in range(2):
                        nc.tensor.matmul(
                            gm[:, :], tw[L - 1][:, f, m * 128 : (m + 1) * 128],
                            arT[:, f, :], start=(f == 0), stop=(f == 1),
                        )
                    nc.scalar.activation(gT[:, m, :], gm[:, :], AF.Copy)
            else:
                nc.scalar.activation(gsum_s[:, :], gz[:, :], AF.Copy)
                nc.sync.dma_start(ar_in[:, :], gsum_s[:, :])
                nc.gpsimd.collective_compute(
                    "AllReduce", ALU.add, replica_groups=[list(range(C))],
                    ins=[ar_in[:, :].opt()], outs=[ar_out[:, :].opt()],
                )
                nc.sync.dma_start(gmean[:, :], ar_out[:, :])
                for f in range(2):
                    tp = ps.tile([128, 128], FP, tag="tp")
                    nc.tensor.transpose(
                        tp[:, 0:G], gmean[:, f * 128 : (f + 1) * 128],
                        ident[0:G, 0:G],
                    )
                    nc.scalar.activation(gT[:, f, :], tp[:, 0:G], AF.Copy)
            for m in range(8):
                mm = ps.tile([128, G], FP, tag="mm")
                for k in range(2):
                    nc.tensor.matmul(
                        mm[:, :], w1s[:, k, m * 128 : (m + 1) * 128], gT[:, k, :],
                        start=(k == 0), stop=(k == 1),
                    )
                lt = sb2.tile([128, G], FP, tag="lk")
                nc.vector.tensor_scalar_mul(lt[:, :], mm[:, :], NEG)
                nc.vector.tensor_max(h1T[:, m, :], mm[:, :], lt[:, :])
            for m in range(4):
                mm = ps.tile([128, G], FP, tag="mm")
                for k in range(8):
                    nc.tensor.matmul(
                        mm[:, :], w2s[:, k, m * 128 : (m + 1) * 128], h1T[:, k, :],
                        start=(k == 0), stop=(k == 7),
                    )
                lt = sb2.tile([128, G], FP, tag="lk")
                nc.vector.tensor_scalar_mul(lt[:, :], mm[:, :], NEG)
                nc.vector.tensor_max(h2T[:, m, :], mm[:, :], lt[:, :])
            fin = ps.tile([1, G], FP, tag="mm")
            for k in range(4):
                nc.tensor.matmul(
                    fin[:, :], w3s[:, k, :], h2T[:, k, :],
                    start=(k == 0), stop=(k == 3),
                )
            nc.scalar.activation(outs[:, :], fin[:, :], AF.Copy)
            nc.sync.dma_start(out_p[:, :], outs[:, :])

    nc.compile()
    return nc


def _prepare(inputs):
    src = inputs["src"]
    dst = inputs["dst"]
    gid = inputs["graph_id"]
    idx_tables, inv, P, K, counts = _host_prep(src, dst, gid)
    uniform_deg = bool((counts == counts[0]).all())
    sum_w = np.asarray(inputs["sum_w"], np.float64)
    w0_sign = [1 if sum_w[i, 0] > 0 else 0 for i in range(L)]
    w1_vals = tuple(float(np.float32(v)) for v in sum_w.flatten())

    for k in ("node_emb_b", "emb_b", "layer_bias", "trans_b",
              "mlp_b1", "mlp_b2", "mlp_b3"):
        assert not np.any(np.asarray(inputs[k])), f"nonzero bias {k} unsupported"

    # relu scale table: inv-degree with w0 folded in (when w0 > 0)
    scl = np.zeros((C, 128, L * NT), np.float32)
    for i in range(L):
        f = sum_w[i, 0] if w0_sign[i] > 0 else 1.0
        scl[:, :, i * NT : (i + 1) * NT] = (inv * f).astype(np.float32)

    def shape_w(w):  # [KR, M] -> [128, KR/128, M]
        KR, M = w.shape
        return np.ascontiguousarray(
            w.reshape(KR // 128, 128, M).transpose(1, 0, 2)
        ).astype(np.float32)

    node_h = np.asarray(inputs["node_h"], np.float32)
    nembW = np.asarray(inputs["node_emb_W"], np.float64)
    embW = np.asarray(inputs["emb_W"], np.float64)
    trnW = np.asarray(inputs["trans_W"], np.float64)
    # host-side weight folds
    W0 = (nembW @ embW[0]).astype(np.float32)                      # [IN, H]
    wcW = np.stack([shape_w(trnW[i] @ embW[i + 1]) for i in range(L - 1)])
    trn_sh = np.stack([shape_w(trnW[i]) for i in range(L)])

    b16 = ml_dtypes.bfloat16
    common = {
        "ident": np.eye(128, dtype=np.float32),
        "nembW": nembW.astype(b16),
        "W0": W0.astype(b16),
        "wcW": wcW.astype(b16),
        "trnW": trn_sh,
        "mlpW1": shape_w(np.asarray(inputs["mlp_W1"], np.float32)),
        "mlpW2": shape_w(np.asarray(inputs["mlp_W2"], np.float32)),
        "mlpW3": shape_w(np.asarray(inputs["mlp_W3"], np.float32)),
    }
    nhT_full = np.zeros((C, IN, NPAD), ml_dtypes.bfloat16)
    for c in range(C):
        nhT_full[c, :, :NLOC] = node_h[c * NLOC : (c + 1) * NLOC].T
    in_maps = []
    for c in range(C):
        m = dict(common)
        m["node_hT"] = np.ascontiguousarray(nhT_full[c])
        m["gidx"] = idx_tables[c]
        m["scl"] = np.ascontiguousarray(scl[c])
        m["Pmat"] = np.ascontiguousarray(P[c].astype(ml_dtypes.bfloat16))
        in_maps.append(m)
    return in_maps, K, uniform_deg, w0_sign, w1_vals


def kernel(_trace=False, **inputs) -> np.ndarray:
    in_maps, K, uniform_deg, w0_sign, w1_vals = _prepare(inputs)
    key = (K, uniform_deg, tuple(w0_sign), w1_vals, L4_TRICK, AG_PIECES, FEAT_F8)
    if key not in _CACHE:
        _CACHE[key] = _build(K, uniform_deg, w0_sign, w1_vals)
    nc = _CACHE[key]
    res = run_bass_kernel_spmd(
        nc, in_maps, core_ids=list(range(C)), trace=_trace
    )
    out = np.asarray(res.results[0]["out"]).reshape(G).astype(np.float32)
    if _trace:
        kernel.last_exec_time_ns = res.exec_time_ns
        kernel.last_results = res
    return out


# revision 36
# speedup vs baseline: 1.0577x; 1.0577x over previous
"""Distributed Trainium2 Bass kernel for nn_AdjGNN (GNN message passing).

Strategy (8 NeuronCores, SPMD):
  - Shard nodes (and their incident edges via dst ownership) evenly: core c
    owns nodes [2500c, 2500c+2500).
  - Per GCN layer: each core computes h = z @ (trans_W @ emb_W_next) for its
    nodes (weights folded host-side), casts to bf16 and AllGathers h across
    the 8 cores into a per-core DRAM replica (halo exchange; src indices are
    uniform-random so the halo is the full node set).  The x-residual
    transform (x' = z @ trans_W) is deferred into the AllGather window so the
    collective starts as early as possible and the PE keeps busy during it.
  - segment_sum(h[src], dst): per-edge rows are fetched with dma_gather
    (one 512B descriptor per edge, four SWDGE queues) into SBUF laid out so
    block k holds the k-th edge of 256 consecutive dsts; the 16:1 in-degree
    reduction is 16 PSUM-accumulating identity matmuls on the TensorEngine
    (bf16 in, fp32 accumulate).  Chunks are fetched as 2 (first chunks: 4)
    sub-gathers so the PE can start reducing before the whole chunk lands.
  - Degree norm + ReLU on ACT; weighted residual fused into one DVE op;
    transform matmuls run in bf16 from PE-transposed z.
  - Readout uses linearity: mean_nodes(z @ trans_W) = (P @ z) @ trans_W, so
    the last layer only accumulates (P @ z)^T per chunk ([256, 100]); the
    final transform runs once on the AllReduced [256, 100] result, then the
    small MLP head (bf16) on every core.

The graph structure (src/dst/graph_id) is known when kernel() is called, so
all index tables (gather indices, inverse degrees, readout matrix) and the
layer weight folds are precomputed host-side.
"""

import ml_dtypes
import numpy as np

import concourse.bacc as bacc
import concourse.mybir as mybir
import concourse.tile as tile
from concourse.bass_utils import run_bass_kernel_spmd

FP = mybir.dt.float32
BF = mybir.dt.bfloat16
F8 = mybir.dt.float8e4
I16 = mybir.dt.int16
AF = mybir.ActivationFunctionType
ALU = mybir.AluOpType

N, E, G = 20000, 320000, 100
IN, H, L = 74, 256, 5
M1, M2 = 1024, 512
NEG = 0.01
C = 8
NLOC = N // C              # 2500 nodes per core
NT = (NLOC + 127) // 128   # 20 node tiles per core
NPAD = NT * 128            # 2560
CH = NT // 2               # chunks of 256 dsts
NFULL = NPAD * C           # 20480

_CACHE = {}
L4_TRICK = True   # readout via (P z) @ W after AllReduce
AG_PIECES = 1     # split each AllGather into row-pieces (1 = single Shared AG)
PIECE = NPAD // AG_PIECES
AG_TRIG_CH = 8    # chunk before whose gathers the first piece is triggered
FEAT_F8 = True    # h features in fp8-e4m3 (halves gather + AllGather bytes)
HT = F8 if FEAT_F8 else BF


def _host_prep(src, dst, gid):
    """Index-only host preprocessing -> per-core gather tables etc."""
    src = np.asarray(src).astype(np.int64)
    dst = np.asarray(dst).astype(np.int64)
    gid = np.asarray(gid).astype(np.int64)

    order = np.argsort(dst, kind="stable")
    src_s, dst_s = src[order], dst[order]
    counts = np.bincount(dst_s, minlength=N)
    starts = np.zeros(N + 1, np.int64)
    np.cumsum(counts, out=starts[1:])
    K = int(counts.max())

    # h_full layout: [piece, rank, PIECE rows] so each AllGather piece is a
    # contiguous block.  Pad gather slots point at row 0 (results for those
    # dsts are zeroed by scl=0, garbage is fine).
    loc = src_s % NLOC
    grow_src = (loc // PIECE) * (C * PIECE) + (src_s // NLOC) * PIECE + (loc % PIECE)

    idx_tables = []
    for c in range(C):
        flat = np.zeros(CH * K * 256, np.int64)
        base = c * NLOC
        for l in range(NLOC):
            n = base + l
            s0, d = starts[n], counts[n]
            ch, r = l // 256, l % 256
            pos0 = ch * K * 256 + r
            flat[pos0 : pos0 + 256 * d : 256] = grow_src[s0 : s0 + d]
        w = flat.reshape(-1, 16).T.astype(np.int16)   # [16, CH*K*16]
        idx_tables.append(np.ascontiguousarray(np.tile(w, (8, 1))))

    inv = np.zeros((C, 128, NT), np.float64)
    for c in range(C):
        cnt = counts[c * NLOC : (c + 1) * NLOC].astype(np.float64)
        iv = np.zeros(NPAD, np.float64)
        iv[:NLOC] = 1.0 / np.maximum(cnt, 1.0)
        inv[c] = iv.reshape(NT, 128).T

    gcnt = np.bincount(gid, minlength=G).astype(np.float64)
    gcnt = np.maximum(gcnt, 1.0)
    P = np.zeros((C, 128, NT, G), np.float32)
    for c in range(C):
        for l in range(NLOC):
            n = c * NLOC + l
            P[c, l % 128, l // 128, gid[n]] = 1.0 / gcnt[gid[n]]

    return idx_tables, inv, P, K, counts


def _build(K, uniform_deg, w0_sign, w1_vals):
    """Build the Bacc graph. w0_sign[i]: +1 fold w0 into relu scale, else 0.
    w1_vals: flattened sum_w baked as immediates."""
    nc = bacc.Bacc("TRN2", debug=False, num_devices=C, num_swdge_queues=4)

    nh_p = nc.declare_dram_parameter("node_hT", [IN, NPAD], BF, isOutput=False)
    idx_p = nc.declare_dram_parameter("gidx", [128, CH * K * 16], I16, isOutput=False)
    scl_p = nc.declare_dram_parameter("scl", [128, L * NT], FP, isOutput=False)
    P_p = nc.declare_dram_parameter("Pmat", [128, NT, G], BF, isOutput=False)
    id_p = nc.declare_dram_parameter("ident", [128, 128], FP, isOutput=False)
    nemb_p = nc.declare_dram_parameter("nembW", [IN, H], BF, isOutput=False)
    w0_p = nc.declare_dram_parameter("W0", [IN, H], BF, isOutput=False)
    wc_p = nc.declare_dram_parameter("wcW", [L - 1, 128, 2, H], BF, isOutput=False)
    trn_p = nc.declare_dram_parameter("trnW", [L, 128, 2, H], FP, isOutput=False)
    w1_p = nc.declare_dram_parameter("mlpW1", [128, 2, M1], FP, isOutput=False)
    w2_p = nc.declare_dram_parameter("mlpW2", [128, 8, M2], FP, isOutput=False)
    w3_p = nc.declare_dram_parameter("mlpW3", [128, 4, 1], FP, isOutput=False)
    out_p = nc.declare_dram_parameter("out", [1, G], FP, isOutput=True)

    with tile.TileContext(nc) as tc:
        with (
            tc.tile_pool(name="dram", bufs=1, space="DRAM") as dram,
            tc.tile_pool(name="sb", bufs=1) as sb,
            tc.tile_pool(name="sb2", bufs=2) as sb2,
            tc.tile_pool(name="ps", bufs=2, space="PSUM") as ps,
        ):
            ag_in = [dram.tile([NPAD, H], HT, name=f"ag_in{i}") for i in range(L)]
            h_space = {"addr_space": "Shared"} if AG_PIECES == 1 else {}
            h_full = [
                dram.tile([NFULL, H], HT, name=f"h_full{i}", **h_space)
                for i in range(L)
            ]
            if L4_TRICK:
                ar_in = dram.tile([128, 2 * G], FP)
                ar_out = dram.tile([128, 2 * G], FP, addr_space="Shared")
            else:
                ar_in = dram.tile([G, H], FP)
                ar_out = dram.tile([G, H], FP, addr_space="Shared")

            ident = sb.tile([128, 128], FP)
            identb = sb.tile([128, 128], BF)
            identh = sb.tile([128, 128], HT)
            idxs = sb.tile([128, CH * K * 16], I16)
            scl = sb.tile([128, L * NT], FP)
            nembb = sb.tile([IN, H], BF)
            w0b = sb.tile([IN, H], BF)
            tw = [sb.tile([128, 2, H], FP, name=f"tw{i}") for i in range(L)]
            wc = [sb.tile([128, 2, H], BF, name=f"wc{i}") for i in range(L - 1)]
            nhT = sb.tile([IN, NT, 128], BF)
            x_a = sb.tile([128, NT, H], FP)
            x_b = sb.tile([128, NT, H], FP)
            zTall = sb.tile([128, CH, 2, H], BF)
            w1s = sb.tile([128, 2, M1], FP)
            w2s = sb.tile([128, 8, M2], FP)
            w3s = sb.tile([128, 4, 1], FP)
            Psb = sb.tile([128, NT, G], BF)
            if not L4_TRICK:
                Psf = sb.tile([128, NT, G], FP)
                gsum_s = sb.tile([G, H], FP)
                gmean = sb.tile([G, H], FP)
            gzs = sb.tile([128, 2, G], FP)
            arT = sb.tile([128, 2, G], FP)
            gT = sb.tile([128, 2, G], FP)
            h1T = sb.tile([128, 8, G], FP)
            h2T = sb.tile([128, 4, G], FP)
            outs = sb.tile([1, G], FP)

            nc.sync.dma_start(ident[:, :], id_p[:, :])
            nc.scalar.activation(identb[:, :], ident[:, :], AF.Copy)
            nc.scalar.activation(identh[:, :], ident[:, :], AF.Copy)
            nc.sync.dma_start(idxs[:, :], idx_p[:, :])
            nc.sync.dma_start(scl[:, :], scl_p[:, :])
            nc.sync.dma_start(w3s[:, :, :], w3_p[:, :, :])
            # bf16 weights arrive host-precast; plain HWDGE loads
            nc.sync.dma_start(
                nhT[:, :, :].rearrange("p t f -> p (t f)"), nh_p[:, :]
            )
            nc.sync.dma_start(w0b[:, :], w0_p[:, :])
            nc.sync.dma_start(nembb[:, :], nemb_p[:, :])
            nc.sync.dma_start(Psb[:, :, :], P_p[:, :, :])
            for i in range(L):
                nc.sync.dma_start(tw[i][:, :, :], trn_p[i, :, :, :])
            for i in range(L - 1):
                nc.sync.dma_start(wc[i][:, :, :], wc_p[i, :, :, :])
            nc.sync.dma_start(w1s[:, :, :], w1_p[:, :, :])
            nc.sync.dma_start(w2s[:, :, :], w2_p[:, :, :])

            # pad rows [NLOC, NPAD) of each ag_in are shipped by the AllGather
            # but never written by the layer body — zero them once
            zb = sb.tile([128, H], HT)
            nc.vector.memset(zb[:, :], 0.0)
            for i in range(L):
                nc.sync.dma_start(ag_in[i][NLOC:NPAD, :], zb[0 : NPAD - NLOC, :])

            # ---- front: h0 = node_h @ (node_emb_W @ emb_W[0]) per tile
            # (node_h arrives host-pretransposed as [IN, NPAD])
            for t in range(NT):
                rows = min(128, NLOC - t * 128)
                h0 = ps.tile([128, H], FP, tag="mm" if t % 2 else "msg")
                nc.tensor.matmul(h0[:, :], nhT[:, t, :], w0b[:, :], start=True, stop=True)
                hs = sb2.tile([128, H], HT, tag="hst")
                nc.scalar.activation(hs[:, :], h0[:, :], AF.Copy)
                nc.sync.dma_start(ag_in[0][t * 128 : t * 128 + rows, :], hs[0:rows, :])
                if (t + 1) % (PIECE // 128) == 0:
                    p = (t + 1) // (PIECE // 128) - 1
                    nc.gpsimd.collective_compute(
                        "AllGather", ALU.bypass, replica_groups=[list(range(C))],
                        ins=[ag_in[0][p * PIECE : (p + 1) * PIECE, :].opt()],
                        outs=[
                            h_full[0][p * PIECE * C : (p + 1) * PIECE * C, :].opt()
                        ],
                    )

            # x0 = node_h @ node_emb_W (overlaps the first AllGather)
            for t in range(NT):
                x0 = ps.tile([128, H], FP, tag="mm" if t % 2 else "msg")
                nc.tensor.matmul(x0[:, :], nhT[:, t, :], nembb[:, :], start=True, stop=True)
                nc.scalar.activation(x_a[:, t, :], x0[:, :], AF.Copy)

            x_cur, x_nxt = x_a, x_b
            gz = None
            gq = 0
            for i in range(L):
                last = i == L - 1
                # -- loop 1: everything that gates the next AllGather --------
                for ch in range(CH):
                    t0 = 2 * ch
                    if i < L - 1 and AG_PIECES == 2 and ch == AG_TRIG_CH:
                        nc.gpsimd.collective_compute(
                            "AllGather", ALU.bypass,
                            replica_groups=[list(range(C))],
                            ins=[ag_in[i + 1][0:PIECE, :].opt()],
                            outs=[h_full[i + 1][0 : PIECE * C, :].opt()],
                        )
                    # sub-split gathers: first chunks 4-way so the PE reduction
                    # starts early after the layer's h_full is ready
                    nsub = 4
                    span = K // nsub
                    gbs = []
                    for sg in range(nsub):
                        if nsub == 2:
                            gsub = sb2.tile(
                                [128, span * 2 * H], HT, tag=f"gh{sg}", bufs=3
                            )
                        else:
                            gsub = sb2.tile(
                                [128, span * 2 * H], HT, tag=f"gs{sg}", bufs=3
                            )
                        nc.gpsimd.dma_gather(
                            gsub[:, 0 : span * 2 * H]
                            .rearrange("p (b e) -> p b e", e=H),
                            h_full[i][:, :],
                            idxs[:, (ch * K + sg * span) * 16 : (ch * K + (sg + 1) * span) * 16],
                            span * 256, span * 256, H,
                            single_packet=False,
                            queue_num=gq % 4,
                        )
                        gq += 1
                        gbs.append(gsub)
                    # PE reduces edge-blocks 0..NPE-1; DVE pairwise-sums the
                    # rest into a bf16 partial merged by one identity matmul
                    NPE = K // 2
                    msg = ps.tile([128, 2 * H], FP, tag="msg")
                    for k in range(NPE):
                        gsub = gbs[k // span]
                        kk = k % span
                        nc.tensor.matmul(
                            msg[:, :], identh[:, :],
                            gsub[:, 2 * kk * H : (2 * kk + 2) * H],
                            start=(k == 0), stop=False,
                        )

                    def blk(k):
                        g = gbs[k // span]
                        kk = k % span
                        return g[:, 2 * kk * H : (2 * kk + 2) * H]

                    pA = sb2.tile([128, 2 * H], BF, tag="pA")
                    pB = sb2.tile([128, 2 * H], BF, tag="pB")
                    nc.vector.tensor_add(pA[:, :], blk(NPE), blk(NPE + 1))
                    nc.vector.tensor_add(pB[:, :], blk(NPE + 2), blk(NPE + 3))
                    nc.vector.tensor_add(pA[:, :], pA[:, :], pB[:, :])
                    nc.vector.tensor_add(pB[:, :], blk(NPE + 4), blk(NPE + 5))
                    nc.vector.tensor_add(pA[:, :], pA[:, :], pB[:, :])
                    nc.vector.tensor_add(pB[:, :], blk(NPE + 6), blk(NPE + 7))
                    nc.vector.tensor_add(pA[:, :], pA[:, :], pB[:, :])
                    nc.tensor.matmul(
                        msg[:, :], identb[:, :], pA[:, :],
                        start=False, stop=True,
                    )
                    r = sb2.tile([128, 2, H], FP, tag="rt")
                    if uniform_deg:
                        nc.scalar.activation(
                            r[:, :, :], msg[:, :].rearrange("p (s e) -> p s e", e=H),
                            AF.Relu,
                            scale=scl[:, i * NT + t0 : i * NT + t0 + 1],
                        )
                    else:
                        for s in range(2):
                            nc.scalar.activation(
                                r[:, s, :], msg[:, s * H : (s + 1) * H],
                                AF.Relu,
                                scale=scl[:, i * NT + t0 + s : i * NT + t0 + s + 1],
                            )
                    if w0_sign[i] <= 0:
                        nc.vector.tensor_scalar_mul(r[:, :, :], r[:, :, :], w1_vals[2 * i])
                    # z = x*w1 + r fused on DVE, bf16 out for the transform path
                    z = sb2.tile([128, 2, H], BF, tag="zt")
                    nc.vector.scalar_tensor_tensor(
                        z[:, :, :], x_cur[:, t0 : t0 + 2, :], w1_vals[2 * i + 1],
                        r[:, :, :], ALU.mult, ALU.add,
                    )
                    if last and L4_TRICK:
                        # readout: accumulate (P @ z)^T = z^T P directly; the
                        # trans_W transform is applied after the AllReduce
                        if gz is None:
                            gz = [
                                ps.tile([128, G], FP, tag=f"gz{f}", bufs=1,
                                        name=f"gz{f}")
                                for f in range(2)
                            ]
                        for s in range(2):
                            for f in range(2):
                                nc.tensor.matmul(
                                    gz[f][:, :],
                                    z[:, s, f * 128 : (f + 1) * 128],
                                    Psb[:, t0 + s, :],
                                    start=(ch == 0 and s == 0),
                                    stop=(ch == CH - 1 and s == 1),
                                    skip_group_check=True,
                                )
                        continue
                    for s in range(2):
                        for f in range(2):
                            tpb = ps.tile([128, 128], BF, tag="tp")
                            nc.tensor.transpose(
                                tpb[:, :], z[:, s, f * 128 : (f + 1) * 128],
                                identb[:, :],
                            )
                            nc.vector.tensor_copy(
                                zTall[:, ch, f, s * 128 : (s + 1) * 128], tpb[:, :]
                            )
                    if not last:
                        hn = ps.tile([128, 2 * H], FP, tag="mm")
                        for s in range(2):
                            for f in range(2):
                                nc.tensor.matmul(
                                    hn[:, s * H : (s + 1) * H],
                                    zTall[:, ch, f, s * 128 : (s + 1) * 128],
                                    wc[i][:, f, :], start=(f == 0), stop=(f == 1),
                                )
                        hs = sb2.tile([128, 2 * H], HT, tag="hst")
                        nc.scalar.activation(hs[:, :], hn[:, :], AF.Copy)
                        for s in range(2):
                            t = t0 + s
                            rows = min(128, NLOC - t * 128)
                            if rows > 0:
                                nc.sync.dma_start(
                                    ag_in[i + 1][t * 128 : t * 128 + rows, :],
                                    hs[0:rows, s * H : (s + 1) * H],
                                )
                    else:
                        # L4_TRICK=False: baseline-style x transform + readout
                        xn = ps.tile([128, 2 * H], FP, tag="mm")
                        for s in range(2):
                            for f in range(2):
                                nc.tensor.matmul(
                                    xn[:, s * H : (s + 1) * H],
                                    zTall[:, ch, f, s * 128 : (s + 1) * 128],
                                    tw[i][:, f, :], start=(f == 0), stop=(f == 1),
                                )
                        nc.scalar.activation(x_nxt[:, t0 : t0 + 2, :], xn[:, :], AF.Copy)
                        if gz is None:
                            gz = ps.tile([G, H], FP, tag="gz", bufs=1)
                        for s in range(2):
                            nc.tensor.matmul(
                                gz[:, :], Psf[:, t0 + s, :], x_nxt[:, t0 + s, :],
                                start=(ch == 0 and s == 0),
                                stop=(ch == CH - 1 and s == 1),
                                skip_group_check=True,
                            )
                if last:
                    break
                p0 = PIECE * (AG_PIECES - 1)
                nc.gpsimd.collective_compute(
                    "AllGather", ALU.bypass, replica_groups=[list(range(C))],
                    ins=[ag_in[i + 1][p0:NPAD, :].opt()],
                    outs=[h_full[i + 1][p0 * C : NFULL, :].opt()],
                )
                # -- loop 2: x' = z @ trans_W, runs inside the AllGather window
                for ch in range(CH):
                    t0 = 2 * ch
                    zT32 = sb2.tile([128, 2, H], FP, tag="z32")
                    for f in range(2):
                        nc.scalar.activation(
                            zT32[:, f, :], zTall[:, ch, f, :], AF.Copy
                        )
                    xn = ps.tile([128, 2 * H], FP, tag="mm")
                    for s in range(2):
                        for f in range(2):
                            nc.tensor.matmul(
                                xn[:, s * H : (s + 1) * H],
                                zT32[:, f, s * 128 : (s + 1) * 128],
                                tw[i][:, f, :], start=(f == 0), stop=(f == 1),
                            )
                    nc.scalar.activation(x_nxt[:, t0 : t0 + 2, :], xn[:, :], AF.Copy)
                x_cur, x_nxt = x_nxt, x_cur

            # ---- readout tail
            if L4_TRICK:
                for f in range(2):
                    nc.scalar.activation(gzs[:, f, :], gz[f][:, :], AF.Copy)
                nc.sync.dma_start(ar_in[:, :], gzs[:, :, :])
                nc.gpsimd.collective_compute(
                    "AllReduce", ALU.add, replica_groups=[list(range(C))],
                    ins=[ar_in[:, :].opt()], outs=[ar_out[:, :].opt()],
                )
                nc.sync.dma_start(arT[:, :, :], ar_out[:, :])
                # gmean^T = trans_W[L-1]^T @ (P z)^T
                for m in range(2):
                    gm = ps.tile([128, G], FP, tag="mm")
                    for f in range(2):
                        nc.tensor.matmul(
                            gm[:, :], tw[L - 1][:, f, m * 128 : (m + 1) * 128],
                            arT[:, f, :], start=(f == 0), stop=(f == 1),
                        )
                    nc.scalar.activation(gT[:, m, :], gm[:, :], AF.Copy)
            else:
                nc.scalar.activation(gsum_s[:, :], gz[:, :], AF.Copy)
                nc.sync.dma_start(ar_in[:, :], gsum_s[:, :])
                nc.gpsimd.collective_compute(
                    "AllReduce", ALU.add, replica_groups=[list(range(C))],
                    ins=[ar_in[:, :].opt()], outs=[ar_out[:, :].opt()],
                )
                nc.sync.dma_start(gmean[:, :], ar_out[:, :])
                for f in range(2):
                    tp = ps.tile([128, 128], FP, tag="tp")
                    nc.tensor.transpose(
                        tp[:, 0:G], gmean[:, f * 128 : (f + 1) * 128],
                        ident[0:G, 0:G],
                    )
                    nc.scalar.activation(gT[:, f, :], tp[:, 0:G], AF.Copy)
            for m in range(8):
                mm = ps.tile([128, G], FP, tag="mm")
                for k in range(2):
                    nc.tensor.matmul(
                        mm[:, :], w1s[:, k, m * 128 : (m + 1) * 128], gT[:, k, :],
                        start=(k == 0), stop=(k == 1),
                    )
                lt = sb2.tile([128, G], FP, tag="lk")
                nc.vector.tensor_scalar_mul(lt[:, :], mm[:, :], NEG)
                nc.vector.tensor_max(h1T[:, m, :], mm[:, :], lt[:, :])
            for m in range(4):
                mm = ps.tile([128, G], FP, tag="mm")
                for k in range(8):
                    nc.tensor.matmul(
                        mm[:, :], w2s[:, k, m * 128 : (m + 1) * 128], h1T[:, k, :],
                        start=(k == 0), stop=(k == 7),
                    )
                lt = sb2.tile([128, G], FP, tag="lk")
                nc.vector.tensor_scalar_mul(lt[:, :], mm[:, :], NEG)
                nc.vector.tensor_max(h2T[:, m, :], mm[:, :], lt[:, :])
            fin = ps.tile([1, G], FP, tag="mm")
            for k in range(4):
                nc.tensor.matmul(
                    fin[:, :], w3s[:, k, :], h2T[:, k, :],
                    start=(k == 0), stop=(k == 3),
                )
            nc.scalar.activation(outs[:, :], fin[:, :], AF.Copy)
            nc.sync.dma_start(out_p[:, :], outs[:, :])

    nc.compile()
    return nc


def _prepare(inputs):
    src = inputs["src"]
    dst = inputs["dst"]
    gid = inputs["graph_id"]
    idx_tables, inv, P, K, counts = _host_prep(src, dst, gid)
    uniform_deg = bool((counts == counts[0]).all())
    sum_w = np.asarray(inputs["sum_w"], np.float64)
    w0_sign = [1 if sum_w[i, 0] > 0 else 0 for i in range(L)]
    w1_vals = tuple(float(np.float32(v)) for v in sum_w.flatten())

    for k in ("node_emb_b", "emb_b", "layer_bias", "trans_b",
              "mlp_b1", "mlp_b2", "mlp_b3"):
        assert not np.any(np.asarray(inputs[k])), f"nonzero bias {k} unsupported"

    # relu scale table: inv-degree with w0 folded in (when w0 > 0)
    scl = np.zeros((C, 128, L * NT), np.float32)
    for i in range(L):
        f = sum_w[i, 0] if w0_sign[i] > 0 else 1.0
        scl[:, :, i * NT : (i + 1) * NT] = (inv * f).astype(np.float32)

    def shape_w(w):  # [KR, M] -> [128, KR/128, M]
        KR, M = w.shape
        return np.ascontiguousarray(
            w.reshape(KR // 128, 128, M).transpose(1, 0, 2)
        ).astype(np.float32)

    node_h = np.asarray(inputs["node_h"], np.float32)
    nembW = np.asarray(inputs["node_emb_W"], np.float64)
    embW = np.asarray(inputs["emb_W"], np.float64)
    trnW = np.asarray(inputs["trans_W"], np.float64)
    # host-side weight folds
    W0 = (nembW @ embW[0]).astype(np.float32)                      # [IN, H]
    wcW = np.stack([shape_w(trnW[i] @ embW[i + 1]) for i in range(L - 1)])
    trn_sh = np.stack([shape_w(trnW[i]) for i in range(L)])

    b16 = ml_dtypes.bfloat16
    common = {
        "ident": np.eye(128, dtype=np.float32),
        "nembW": nembW.astype(b16),
        "W0": W0.astype(b16),
        "wcW": wcW.astype(b16),
        "trnW": trn_sh,
        "mlpW1": shape_w(np.asarray(inputs["mlp_W1"], np.float32)),
        "mlpW2": shape_w(np.asarray(inputs["mlp_W2"], np.float32)),
        "mlpW3": shape_w(np.asarray(inputs["mlp_W3"], np.float32)),
    }
    nhT_full = np.zeros((C, IN, NPAD), ml_dtypes.bfloat16)
    for c in range(C):
        nhT_full[c, :, :NLOC] = node_h[c * NLOC : (c + 1) * NLOC].T
    in_maps = []
    for c in range(C):
        m = dict(common)
        m["node_hT"] = np.ascontiguousarray(nhT_full[c])
        m["gidx"] = idx_tables[c]
        m["scl"] = np.ascontiguousarray(scl[c])
        m["Pmat"] = np.ascontiguousarray(P[c].astype(ml_dtypes.bfloat16))
        in_maps.append(m)
    return in_maps, K, uniform_deg, w0_sign, w1_vals


def kernel(_trace=False, **inputs) -> np.ndarray:
    in_maps, K, uniform_deg, w0_sign, w1_vals = _prepare(inputs)
    key = (K, uniform_deg, tuple(w0_sign), w1_vals, L4_TRICK, AG_PIECES, FEAT_F8)
    if key not in _CACHE:
        _CACHE[key] = _build(K, uniform_deg, w0_sign, w1_vals)
    nc = _CACHE[key]
    res = run_bass_kernel_spmd(
        nc, in_maps, core_ids=list(range(C)), trace=_trace
    )
    out = np.asarray(res.results[0]["out"]).reshape(G).astype(np.float32)
    if _trace:
        kernel.last_exec_time_ns = res.exec_time_ns
        kernel.last_results = res
    return out
